# revision 4
# baseline (speedup 1.0000x reference)
"""ARC quant layer on 8 TRN2 NeuronCores.

out[b,s,o] = quant(x) @ quant(W)^T + (x_outl - quant(x_outl)) @ arcW^T
with quant(v) = round_half_even(8 v) / 8.

Sharding: 4-way on the 8192 flattened batch*seq rows x 2-way on the 4096
out_features (minimizes per-core DMA: 32MB x-shard + 32MB w-shard).

Device math: k = round(8v) is a small integer (|k| <= ~50), exact in bf16,
so the main matmul runs at bf16 TensorE rate with an exactly-integer fp32
PSUM accumulation of kx.kw = 64*(x_q.w_q). The outlier compensation is
accumulated into the same PSUM as (64*x_res) @ arc^T over a zero-padded
K=256, and the single PSUM->SBUF copy applies the 1/64 scale.

Rounding on device: y = fl32(8x + C) with C = 1.5*2^23 gives
y = C + round_half_even(8x) exactly; k = y - C (exact, Sterbenz).
"""

import numpy as np

import concourse.bass as bass
from concourse import bacc
import concourse.mybir as mybir
import concourse.tile as tile
from concourse.bass_utils import run_bass_kernel_spmd

F32 = mybir.dt.float32
BF16 = mybir.dt.bfloat16

ROWS = 8192          # 4*2048 flattened batch*seq
D = 4096             # in_features
O = 4096             # out_features
KO = 204             # num outliers
KOP = 256            # padded outlier contraction dim

RSHARDS = 4          # row shards
FSHARDS = 2          # out_feature shards
R = ROWS // RSHARDS  # 2048 rows per core
F = O // FSHARDS     # 2048 out_features per core

KT = D // 128        # 32 k-tiles
CHUNK = 256          # rows per chunk
NCHUNK = R // CHUNK  # 8
KGRP = 4             # k-tiles quantized per group (one DMA/op covers 4*256)
NGRP = KT // KGRP    # 8

MAGIC = 12582912.0       # 1.5 * 2**23
MAGIC8 = 8.0 * MAGIC

_CACHED_NC = None


def build_nc():
    nc = bacc.Bacc(None)

    xT = nc.declare_dram_parameter("xT", [D, R], F32, isOutput=False)
    wT = nc.declare_dram_parameter("wT", [D, F], F32, isOutput=False)
    xoT = nc.declare_dram_parameter("xoT", [KOP, R], F32, isOutput=False)
    arcT = nc.declare_dram_parameter("arcT", [KOP, F], F32, isOutput=False)
    out_ext = nc.declare_dram_parameter("out", [R, F], F32, isOutput=True)

    Copy = mybir.ActivationFunctionType.Copy
    sub = mybir.AluOpType.subtract
    mult = mybir.AluOpType.mult

    with tile.TileContext(nc) as tc:
        with (
            tc.tile_pool(name="kw", bufs=KT) as kw_pool,
            tc.tile_pool(name="karc", bufs=2) as karc_pool,
            tc.tile_pool(name="r64", bufs=2) as r64_pool,
            tc.tile_pool(name="kx", bufs=12) as kx_pool,
            tc.tile_pool(name="stage", bufs=4) as stage_pool,
            tc.tile_pool(name="ystage", bufs=2) as y_pool,
            tc.tile_pool(name="outp", bufs=2) as out_pool,
            tc.tile_pool(name="psum", bufs=2, space="PSUM") as psum_pool,
        ):
            # ---- prologue: quantize W into resident bf16 k-tiles ----
            kw = []
            for k in range(KT):
                kwt = kw_pool.tile([128, F], BF16, tag="kw")
                for h in range(2):
                    hs = slice(h * 1024, (h + 1) * 1024)
                    wst = stage_pool.tile([128, 1024], F32, tag="stage")
                    nc.sync.dma_start(out=wst, in_=wT[k * 128:(k + 1) * 128, hs])
                    yw = y_pool.tile([128, 1024], F32, tag="y")
                    nc.vector.tensor_scalar(
                        out=yw, in0=wst, scalar1=8.0, scalar2=MAGIC,
                        op0=mult, op1=mybir.AluOpType.add)
                    nc.scalar.activation(kwt[:, hs], yw, Copy, bias=-MAGIC)
                kw.append(kwt)

            # ---- arc weights -> bf16 (zero-padded rows come in as zeros) ----
            karc = []
            for t in range(2):
                kat = karc_pool.tile([128, F], BF16, tag="karc")
                for h in range(2):
                    hs = slice(h * 1024, (h + 1) * 1024)
                    ast = stage_pool.tile([128, 1024], F32, tag="stage")
                    nc.sync.dma_start(out=ast, in_=arcT[t * 128:(t + 1) * 128, hs])
                    nc.vector.tensor_copy(kat[:, hs], ast)
                karc.append(kat)

            # ---- outlier residuals: r64 = 64*x - 8*round(8x)  (bf16) ----
            r64 = []
            for t in range(2):
                rt = r64_pool.tile([128, R], BF16, tag="r64")
                for h in range(2):
                    hs = slice(h * 1024, (h + 1) * 1024)
                    xost = stage_pool.tile([128, 1024], F32, tag="stage")
                    nc.sync.dma_start(out=xost, in_=xoT[t * 128:(t + 1) * 128, hs])
                    yo = y_pool.tile([128, 1024], F32, tag="y")
                    nc.vector.tensor_scalar(
                        out=yo, in0=xost, scalar1=8.0, scalar2=MAGIC,
                        op0=mult, op1=mybir.AluOpType.add)
                    t8k = stage_pool.tile([128, 1024], F32, tag="stage")
                    nc.vector.tensor_scalar(
                        out=t8k, in0=yo, scalar1=8.0, scalar2=MAGIC8,
                        op0=mult, op1=sub)
                    x64 = stage_pool.tile([128, 1024], F32, tag="stage")
                    nc.vector.tensor_scalar_mul(x64, xost, 64.0)
                    nc.vector.tensor_tensor(out=rt[:, hs], in0=x64, in1=t8k, op=sub)
                r64.append(rt)

            # ---- main loop over row chunks ----
            xT_g = xT.rearrange("(g p) r -> p g r", p=128)  # [128, KT, R]
            for ch in range(NCHUNK):
                cs = slice(ch * CHUNK, (ch + 1) * CHUNK)
                kxg = []
                for g in range(NGRP):
                    xg = stage_pool.tile([128, KGRP, CHUNK], F32, tag="stage")
                    nc.sync.dma_start(
                        out=xg, in_=xT_g[:, g * KGRP:(g + 1) * KGRP, cs])
                    yg = y_pool.tile([128, KGRP, CHUNK], F32, tag="y")
                    nc.vector.tensor_scalar(
                        out=yg, in0=xg, scalar1=8.0, scalar2=MAGIC,
                        op0=mult, op1=mybir.AluOpType.add)
                    kxt = kx_pool.tile([128, KGRP, CHUNK], BF16, tag="kx")
                    nc.scalar.activation(kxt, yg, Copy, bias=-MAGIC)
                    kxg.append(kxt)

                for rb in range(CHUNK // 128):
                    rows0 = ch * CHUNK + rb * 128
                    psum = psum_pool.tile([128, F], F32, tag="psum")
                    # outlier compensation first (K=256, zero-padded)
                    for t in range(2):
                        lhsT = r64[t][:, rows0:rows0 + 128]
                        for j in range(F // 512):
                            js = slice(j * 512, (j + 1) * 512)
                            nc.tensor.matmul(
                                psum[:, js], lhsT, karc[t][:, js],
                                start=(t == 0), stop=False)
                    # main quantized matmul
                    for k in range(KT):
                        g, kk = divmod(k, KGRP)
                        lhsT = kxg[g][:, kk, rb * 128:(rb + 1) * 128]
                        for j in range(F // 512):
                            js = slice(j * 512, (j + 1) * 512)
                            nc.tensor.matmul(
                                psum[:, js], lhsT, kw[k][:, js],
                                start=False, stop=(k == KT - 1))
                    # epilogue: scale 1/64, DMA out
                    for h in range(2):
                        hs = slice(h * 1024, (h + 1) * 1024)
                        outt = out_pool.tile([128, 1024], F32, tag="out")
                        nc.scalar.activation(
                            outt, psum[:, hs], Copy, scale=1.0 / 64.0)
                        nc.sync.dma_start(
                            out=out_ext[rows0:rows0 + 128, hs], in_=outt)
    nc.finalize()
    return nc


def prepare_in_maps(x, weight, arc_weight, outlier_indices):
    xf = np.ascontiguousarray(x.reshape(ROWS, D))
    idx = np.asarray(outlier_indices)
    in_maps = []
    for c in range(8):
        rs, fs = c % RSHARDS, c // RSHARDS
        xs = xf[rs * R:(rs + 1) * R]                      # [R, D]
        xT = np.ascontiguousarray(xs.T)                    # [D, R]
        ws = weight[fs * F:(fs + 1) * F]                   # [F, D]
        wT = np.ascontiguousarray(ws.T)                    # [D, F]
        arcT = np.zeros((KOP, F), dtype=np.float32)
        arcT[:KO] = arc_weight[fs * F:(fs + 1) * F].T      # [KO, F]
        xoT = np.zeros((KOP, R), dtype=np.float32)
        xoT[:KO] = xs[:, idx].T                            # [KO, R]
        in_maps.append({
            "xT": xT, "wT": wT,
            "xoT": np.ascontiguousarray(xoT),
            "arcT": np.ascontiguousarray(arcT),
        })
    return in_maps


def assemble(results):
    out = np.empty((ROWS, O), dtype=np.float32)
    for c in range(8):
        rs, fs = c % RSHARDS, c // RSHARDS
        out[rs * R:(rs + 1) * R, fs * F:(fs + 1) * F] = results[c]["out"]
    return out.reshape(4, 2048, 4096)


def kernel(x, weight, arc_weight, outlier_indices):
    global _CACHED_NC
    if _CACHED_NC is None:
        _CACHED_NC = build_nc()
    in_maps = prepare_in_maps(
        np.asarray(x, dtype=np.float32),
        np.asarray(weight, dtype=np.float32),
        np.asarray(arc_weight, dtype=np.float32),
        outlier_indices,
    )
    res = run_bass_kernel_spmd(_CACHED_NC, in_maps, core_ids=list(range(8)))
    return assemble(res.results)
